# revision 1
# baseline (speedup 1.0000x reference)
"""Trainium2 Bass kernel for nn_CropRoi (3D RoI crop + adaptive max pool).

Contract: kernel(**inputs) takes FULL unsharded inputs
  f:         [B=2, C=128, Df=24, Hf=24, Wf=24] float32 feature map
  inputs:    [B, 1, D=96, H=96, W=96] float32 (only shape used)
  proposals: [N=64, 8] float32 (batch, score, center_zyx, side_zyx)
  scale:     scalar int
and returns the FULL output [N, C, 7, 7, 7] float32.

Strategy: shard proposals across the 8 NeuronCores (8 RoIs per core),
replicate f. Each core runs a fully specialized Bass/Tile program
(geometry baked in at build time): per proposal a single HWDGE DMA
loads the crop slab [C=128, Lz, Ly, Wf] into SBUF, then 21 ragged
reduce_max ops (7 per spatial dim) do the separable adaptive max-pool
on the Vector/GpSimd engines, and one DMA stores [C, 343] per RoI.
"""

import sys

if "/opt/trn_rl_repo" not in sys.path:
    sys.path.insert(0, "/opt/trn_rl_repo")

import numpy as np

S_OUT = 7
NEG32 = np.float32(np.finfo(np.float32).min)
N_CORES = 8


# ----------------------------------------------------------------- host geometry
def _bins_1d(L):
    """Adaptive-pool windows for length L split into S_OUT bins.

    Returns (starts, widths) relative to the crop origin. Matches the
    reference's floor/ceil bin edges; for L <= 0 all windows are empty.
    """
    i = np.arange(S_OUT, dtype=np.int64)
    starts = (i * L) // S_OUT
    ends = -((-(i + 1) * L) // S_OUT)
    widths = np.maximum(ends - starts, 0)
    return starts.astype(int), widths.astype(int)


def build_geometry(f_shape, proposals, scale):
    """Mirror the reference's crop-bound computation exactly (float32 ops)."""
    B, C, Df, Hf, Wf = f_shape
    maxd = np.array([Df, Hf, Wf], np.int32)
    p = np.asarray(proposals, np.float32)
    center = p[:, 2:5].astype(np.float32)
    side = p[:, 5:8].astype(np.float32)
    c0f = center - side / np.float32(2.0)
    c1f = c0f + side
    sc = np.float32(scale)
    c0 = np.floor(c0f / sc).astype(np.int32)
    c1 = np.ceil(c1f / sc).astype(np.int32)
    c0 = np.maximum(c0, 0)
    c1 = np.minimum(c1, maxd[None, :])
    b = np.clip(p[:, 0].astype(np.int32), 0, B - 1)

    geoms = []
    for n in range(p.shape[0]):
        L = (c1[n] - c0[n]).astype(int)
        g = {
            "b": int(b[n]),
            "orig": [int(v) for v in c0[n]],
            "L": [int(v) for v in L],
            "empty": bool((L <= 0).any()),
        }
        g["zbins"] = _bins_1d(L[0])
        g["ybins"] = _bins_1d(L[1])
        g["xbins"] = _bins_1d(L[2])
        geoms.append(g)
    return geoms


# ----------------------------------------------------------------- bass program
def build_core_program(f_shape, geoms):
    """One specialized program for one core: len(geoms) proposals."""
    import concourse.bacc as bacc
    import concourse.tile as tile
    import concourse.mybir as mybir

    B, C, Df, Hf, Wf = f_shape
    P = len(geoms)
    nc = bacc.Bacc("TRN2", target_bir_lowering=False, debug=False, num_devices=1)
    f_ap = nc.dram_tensor("f", [B, C, Df, Hf, Wf], mybir.dt.float32,
                          kind="ExternalInput").ap()
    o_ap = nc.dram_tensor("o", [P, C, S_OUT, S_OUT, S_OUT], mybir.dt.float32,
                          kind="ExternalOutput").ap()

    with tile.TileContext(nc) as tc:
        with tc.tile_pool(name="pool", bufs=3) as pool, \
             tc.tile_pool(name="opool", bufs=3) as opool:
            for pi, g in enumerate(geoms):
                yx = opool.tile([C, S_OUT, S_OUT, S_OUT], mybir.dt.float32,
                                tag="yx")
                if g["empty"]:
                    nc.vector.memset(yx[:], float(NEG32))
                    nc.scalar.dma_start(out=o_ap[pi], in_=yx[:])
                    continue
                Lz, Ly, Lx = g["L"]
                zc, yc, xc = g["orig"]
                bidx = g["b"]
                # full-Wf rows keep DMA descriptors contiguous (Ly*Wf*4 bytes)
                crop = pool.tile([C, Lz, Ly, Wf], mybir.dt.float32, tag="crop")
                nc.sync.dma_start(
                    out=crop[:],
                    in_=f_ap[bidx, :, zc:zc + Lz, yc:yc + Ly, :],
                )
                # stage z: [C, Lz, Ly, Wf] -> [C, 7, Ly, Lx]
                yz = pool.tile([C, S_OUT, Ly, Lx], mybir.dt.float32, tag="yz")
                zs, zw = g["zbins"]
                for i in range(S_OUT):
                    src = crop[:, zs[i]:zs[i] + zw[i], 0:Ly, xc:xc + Lx]
                    src = src.rearrange("c z y x -> c y x z")
                    nc.vector.reduce_max(out=yz[:, i], in_=src,
                                         axis=mybir.AxisListType.X)
                # stage y: [C, 7, Ly, Lx] -> [C, 7, 7, Lx]
                yy = pool.tile([C, S_OUT, S_OUT, Lx], mybir.dt.float32, tag="yy")
                ys, yw = g["ybins"]
                for j in range(S_OUT):
                    src = yz[:, :, ys[j]:ys[j] + yw[j], :]
                    src = src.rearrange("c i y x -> c i x y")
                    nc.vector.reduce_max(out=yy[:, :, j], in_=src,
                                         axis=mybir.AxisListType.X)
                # stage x: [C, 7, 7, Lx] -> [C, 7, 7, 7]
                xs, xw = g["xbins"]
                for k in range(S_OUT):
                    src = yy[:, :, :, xs[k]:xs[k] + xw[k]]
                    nc.vector.reduce_max(out=yx[:, :, :, k], in_=src,
                                         axis=mybir.AxisListType.X)
                nc.scalar.dma_start(out=o_ap[pi], in_=yx[:])
    nc.compile()
    return nc


# ----------------------------------------------------------------- jax dispatch
def _make_jit(nc):
    """Mirror bass2jax.run_bass_via_pjrt's single-core body as a reusable jit."""
    import jax
    import concourse.mybir as mybir
    from concourse.bass2jax import (
        _bass_exec_p,
        install_neuronx_cc_hook,
        partition_id_tensor,
    )

    install_neuronx_cc_hook()

    partition_name = nc.partition_id_tensor.name if nc.partition_id_tensor else None
    in_names, out_names, out_avals, zero_outs = [], [], [], []
    for alloc in nc.m.functions[0].allocations:
        if not isinstance(alloc, mybir.MemoryLocationSet):
            continue
        name = alloc.memorylocations[0].name
        if alloc.kind == "ExternalInput":
            if name != partition_name:
                in_names.append(name)
        elif alloc.kind == "ExternalOutput":
            out_names.append(name)
            shape = tuple(alloc.tensor_shape)
            dtype = mybir.dt.np(alloc.dtype)
            out_avals.append(jax.core.ShapedArray(shape, dtype))
            zero_outs.append(np.zeros(shape, dtype))
    n_params = len(in_names)
    all_names = tuple(
        in_names + out_names + ([partition_name] if partition_name else [])
    )
    donate = tuple(range(n_params, n_params + len(out_names)))

    def _body(*args):
        operands = list(args)
        if partition_name is not None:
            operands.append(partition_id_tensor())
        outs = _bass_exec_p.bind(
            *operands,
            out_avals=tuple(out_avals),
            in_names=all_names,
            out_names=tuple(out_names),
            lowering_input_output_aliases=(),
            sim_require_finite=True,
            sim_require_nnan=True,
            nc=nc,
        )
        return tuple(outs)

    fn = jax.jit(_body, donate_argnums=donate, keep_unused=True)
    return fn, list(in_names), list(out_names), zero_outs


def _make_chain_jit(nc, reps):
    """Jit that runs the program `reps` times back-to-back on device,
    chaining the output buffer through, to measure per-execution HW time
    without per-rep host dispatch."""
    import jax
    import concourse.mybir as mybir
    from concourse.bass2jax import (
        _bass_exec_p,
        install_neuronx_cc_hook,
        partition_id_tensor,
    )

    install_neuronx_cc_hook()

    partition_name = nc.partition_id_tensor.name if nc.partition_id_tensor else None
    in_names, out_names, out_avals = [], [], []
    for alloc in nc.m.functions[0].allocations:
        if not isinstance(alloc, mybir.MemoryLocationSet):
            continue
        name = alloc.memorylocations[0].name
        if alloc.kind == "ExternalInput":
            if name != partition_name:
                in_names.append(name)
        elif alloc.kind == "ExternalOutput":
            out_names.append(name)
            out_avals.append(
                jax.core.ShapedArray(tuple(alloc.tensor_shape),
                                     mybir.dt.np(alloc.dtype))
            )
    all_names = tuple(
        in_names + out_names + ([partition_name] if partition_name else [])
    )

    def _step(_i, o, f):
        operands = [f, o]
        if partition_name is not None:
            operands.append(partition_id_tensor())
        (o,) = _bass_exec_p.bind(
            *operands,
            out_avals=tuple(out_avals),
            in_names=all_names,
            out_names=tuple(out_names),
            lowering_input_output_aliases=(),
            sim_require_finite=True,
            sim_require_nnan=True,
            nc=nc,
        )
        return o

    def _body(f, o):
        return jax.lax.fori_loop(0, reps, lambda i, o: _step(i, o, f), o)

    return jax.jit(_body, donate_argnums=(1,), keep_unused=True)


class CompiledKernel:
    """8 specialized per-core programs plus their jitted entry points."""

    def __init__(self, f_shape, geoms):
        import jax

        self.devices = jax.devices()[:N_CORES]
        assert len(self.devices) == N_CORES
        self.per_core = []
        self.ncs = []
        per = len(geoms) // N_CORES
        for k in range(N_CORES):
            nc = build_core_program(f_shape, geoms[k * per:(k + 1) * per])
            self.ncs.append(nc)
            self.per_core.append(_make_jit(nc))

    def run(self, f):
        import jax

        outs = []
        for k, (fn, in_names, _out_names, zero_outs) in enumerate(self.per_core):
            assert in_names == ["f"]
            with jax.default_device(self.devices[k]):
                outs.append(fn(f, *[z.copy() for z in zero_outs]))
        return [np.asarray(o[0]) for o in outs]


def kernel(**inputs):
    f = np.ascontiguousarray(np.asarray(inputs["f"], dtype=np.float32))
    proposals = np.asarray(inputs["proposals"], dtype=np.float32)
    scale = int(np.asarray(inputs["scale"]))
    geoms = build_geometry(f.shape, proposals, scale)
    ck = CompiledKernel(f.shape, geoms)
    kernel.last_compiled = ck  # reused by test.py for benchmarking
    kernel.last_f = f
    parts = ck.run(f)
    return np.concatenate(parts, axis=0)


kernel.last_compiled = None
kernel.last_f = None

